# revision 9
# baseline (speedup 1.0000x reference)
"""MixHop layer (3 hops) on 8 Trainium2 NeuronCores.

out = concat_j [ adj_t^j @ (x @ W_j.T + b_j) ]   for j = 0,1,2

Strategy (destination sharding, one SPMD program on 8 cores):
  - Each core owns 6250 destination rows and the edges pointing into them.
  - Phase B: every core (redundantly) projects x -> [y1 | y2] table [N,256]
    with PE matmuls (x fed pre-transposed).  Phase A: y0 for own shard.
  - Phase C (SpMM1): per destination-block of 128 nodes (degree-balanced on
    host), dma_gather the 1024B table rows of each in-edge source, build the
    one-hot*weight segment matrix S on device (tensor_scalar is_equal+mult
    against an iota tile), segment-sum via PE matmuls accumulated in PSUM.
    Cols 0:128 -> out1 (scattered to output rows), cols 128:256 -> z2 shard.
  - AllGather z2 shards -> full z2 table [N,128].
  - Phase E (SpMM2): same edge structure gathers z2 -> out2.
All per-core variation (indices, segment matrices, scatter rows) is carried
as input data so a single program serves all cores.
"""

import sys

sys.path.insert(0, "/opt/trn_rl_repo")

import heapq
import os

import numpy as np

import concourse.bass as bass
import concourse.tile as tile
from concourse import bacc, mybir
from concourse import bass_utils

P = 128


class Cfg:
    def __init__(self, n_nodes, n_feat, n_cores, k0max, k1max, dt=mybir.dt.float32):
        assert n_nodes % n_cores == 0
        self.N = n_nodes
        self.F = n_feat          # 128
        self.NC = n_cores        # 8
        self.NS = n_nodes // n_cores          # dests per core
        self.NBLK = -(-self.NS // P)          # blocks per core
        if self.NBLK % 2:
            self.NBLK += 1                    # groups of 2 blocks
        self.NG = self.NBLK // 2
        self.GMAX = 8                         # chunks per dma_gather (<=1024 idxs)
        self.G0 = -(-k0max // self.GMAX)
        self.G1 = -(-k1max // self.GMAX)
        self.K0 = k0max                       # window-0 chunks per block
        self.K1 = k1max                       # window-1 chunks per block
        self.K = k0max + k1max
        self.WIN = 32768 if n_nodes > 32768 else max(P, n_nodes // 2)
        self.NPAD = self.NBLK * P             # padded shard rows (incl trash)
        self.dt = dt


def _balanced_blocks(local_dest, ns, nblk):
    """Assign dests 0..ns-1 to nblk blocks of <=P slots, balancing edge counts.
    Returns (block_of[ns], pos_of[ns], ids[P, nblk] local out rows)."""
    deg = np.bincount(local_dest, minlength=ns)
    order = np.argsort(-deg, kind="stable")
    heap = [(0, 0, b) for b in range(nblk)]
    heapq.heapify(heap)
    block_of = np.empty(ns, np.int32)
    pos_of = np.empty(ns, np.int32)
    stash = []
    for d in order:
        while True:
            load, cnt, b = heapq.heappop(heap)
            if cnt < P:
                break
            stash.append((load, cnt, b))
        block_of[d] = b
        pos_of[d] = cnt
        heapq.heappush(heap, (load + int(deg[d]), cnt + 1, b))
    # ids: slot p of block b -> local output row (trash rows = ns + p for pads)
    ids = np.empty((P, nblk), np.int32)
    for p in range(P):
        ids[p, :] = ns + p  # trash default (distinct per slot within a block)
    ids[pos_of, block_of] = np.arange(ns, dtype=np.int32)
    return block_of, pos_of, ids


def _precompute_core(r_loc, c_glob, w, cfg):
    """Per-core edge streams.  Returns dict plus per-core chunk maxes."""
    ns, nblk = cfg.NS, cfg.NBLK
    block_of, pos_of, ids = _balanced_blocks(r_loc, ns, nblk)
    b_e = block_of[r_loc]
    dl_e = pos_of[r_loc]
    win_e = (c_glob >= cfg.WIN).astype(np.int64)
    # group edges by (block, window); stable order within group
    order = np.lexsort((np.arange(len(r_loc)), win_e, b_e))
    b_s, win_s, dl_s, c_s, w_s = (
        b_e[order], win_e[order], dl_e[order], c_glob[order], w[order])
    # counts per (block, window)
    key = b_s * 2 + win_s
    cnt = np.bincount(key, minlength=nblk * 2).reshape(nblk, 2)
    k0need = max(1, int(np.ceil(cnt[:, 0].max() / P))) if len(r_loc) else 1
    k1need = max(1, int(np.ceil(cnt[:, 1].max() / P))) if len(r_loc) else 1
    return dict(b=b_s, win=win_s, dl=dl_s, c=c_s, w=w_s, cnt=cnt, ids=ids,
                k0=k0need, k1=k1need)


def _encode_core(pc, cfg):
    """Build device input arrays for one core, given global K0/K1."""
    nblk, K0, K1, K = cfg.NBLK, cfg.K0, cfg.K1, cfg.K
    ng = cfg.NG
    cnt = pc["cnt"]
    # flat idx arrays, one per window, padded to static sizes (pad idx = 0)
    idx0 = np.zeros((nblk, K0 * P), np.int16)
    idx1 = np.zeros((nblk, K1 * P), np.int16)
    meta = np.zeros((P, nblk, K, 2), np.float32)
    # within-group position for every edge
    starts = np.zeros(nblk * 2, np.int64)
    starts[1:] = np.cumsum(cnt.reshape(-1))[:-1]
    key = pc["b"] * 2 + pc["win"]
    iw = np.arange(len(key)) - starts[key]          # index within (b, win)
    b, win, dl, c, w = pc["b"], pc["win"], pc["dl"], pc["c"], pc["w"]
    m0 = win == 0
    idx0[b[m0], iw[m0]] = c[m0].astype(np.int16)
    m1 = ~m0
    idx1[b[m1], iw[m1]] = (c[m1] - cfg.WIN).astype(np.int16)
    kk = np.where(m0, iw // P, K0 + iw // P)        # chunk within block
    meta[iw % P, b, kk, 0] = dl
    meta[iw % P, b, kk, 1] = w
    # per-gather idx encodings [nblk*Gw, 128, GMAX*8] (<=1024 idxs each)
    GM = cfg.GMAX
    def enc(idx, Kw, Gw):
        out = np.zeros((nblk, Gw, P, GM * 8), np.int16)
        for b in range(nblk):
            for g in range(Gw):
                cg = min(GM, Kw - GM * g)
                flat = idx[b, g * GM * P: g * GM * P + cg * P]
                e = flat.reshape(-1, 16).T                    # [16, cg*8]
                out[b, g, :, :cg * 8] = np.tile(e, (8, 1))
        return out.reshape(nblk * Gw * P, GM * 8)
    return dict(
        idx0=enc(idx0, K0, cfg.G0), idx1=enc(idx1, K1, cfg.G1),
        meta=np.ascontiguousarray(meta.reshape(P, nblk * K * 2)),
        ids=np.ascontiguousarray(pc["ids"]),
    )


def _build_program(cfg, phases="ABCDE"):
    N, F, NC = cfg.N, cfg.F, cfg.NC
    NS, NBLK, NG, K0, K1, K = cfg.NS, cfg.NBLK, cfg.NG, cfg.K0, cfg.K1, cfg.K
    W0, W1 = 0, cfg.WIN                      # window bases
    NW0 = min(N, cfg.WIN)
    NW1 = max(0, N - cfg.WIN)
    NSP = NS + P                             # out buf rows incl trash
    NTILE_Y0 = NBLK                          # 128-row tiles for own shard
    NTILE_TAB = -(-N // P)                   # 128-row tiles for full table
    f32 = mybir.dt.float32

    nc = bacc.Bacc("TRN2", target_bir_lowering=False, debug=False,
                   enable_asserts=False, num_devices=NC, num_swdge_queues=4)

    # ---- inputs -----------------------------------------------------------
    xsT = nc.dram_tensor("xsT", [F, NBLK * P], f32, kind="ExternalInput").ap()
    WT = nc.dram_tensor("WT", [3 * F, F], f32, kind="ExternalInput").ap()
    BB = nc.dram_tensor("BB", [3 * P, F], f32, kind="ExternalInput").ap()
    iota_in = nc.dram_tensor("iota", [P, P], f32, kind="ExternalInput").ap()
    GM, G0, G1 = cfg.GMAX, cfg.G0, cfg.G1
    idx0_in = nc.dram_tensor("idx0", [NBLK * G0 * P, GM * 8], mybir.dt.int16,
                             kind="ExternalInput").ap()
    idx1_in = nc.dram_tensor("idx1", [NBLK * G1 * P, GM * 8], mybir.dt.int16,
                             kind="ExternalInput").ap()
    meta_in = nc.dram_tensor("meta", [P, NBLK * K * 2], f32,
                             kind="ExternalInput").ap()
    ids_in = nc.dram_tensor("ids", [P, NBLK], mybir.dt.int32,
                            kind="ExternalInput").ap()

    # ---- outputs / scratch ------------------------------------------------
    y0_buf = nc.dram_tensor("y0", [NBLK * P, F], f32, kind="ExternalOutput").ap()
    out1_buf = nc.dram_tensor("out1", [NSP, F], f32, kind="ExternalOutput").ap()
    out2_buf = nc.dram_tensor("out2", [NSP, F], f32, kind="ExternalOutput").ap()
    tbl_mine = nc.dram_tensor("tbl_mine", [NS, 2 * F], f32, kind="Internal").ap()
    table = nc.dram_tensor("table", [N, 2 * F], f32, kind="Internal",
                           addr_space="Shared").ap()
    z2s = nc.dram_tensor("z2s", [NSP, F], f32, kind="Internal").ap()
    z2t = nc.dram_tensor("z2t", [N, F], f32, kind="Internal",
                         addr_space="Shared").ap()

    with tile.TileContext(nc) as tc:
        with tc.tile_pool(name="const", bufs=1) as cpool:
            iota_t = cpool.tile([P, P], f32)
            nc.sync.dma_start(iota_t[:], iota_in[:])
            ids_t = cpool.tile([P, NBLK], mybir.dt.int32)
            nc.sync.dma_start(ids_t[:], ids_in[:])
            meta_t = cpool.tile([P, NBLK * K * 2], f32)
            nc.sync.dma_start(meta_t[:], meta_in[:])
            wt_t = []
            bb_t = []
            for j in range(3):
                wtj = cpool.tile([F, F], f32, tag=f"wt{j}", name=f"wt{j}")
                bbj = cpool.tile([P, F], f32, tag=f"bb{j}", name=f"bb{j}")
                wt_t.append(wtj)
                bb_t.append(bbj)
            for j in range(3):
                nc.sync.dma_start(wt_t[j][:], WT[j * F:(j + 1) * F, :])
                nc.sync.dma_start(bb_t[j][:], BB[j * P:(j + 1) * P, :])

            # ---- Phase A/B: own-shard projections ------------------------
            # y0 = xs@W0.T+b0 -> y0_buf; [xs@W1.T+b1 | xs@W2.T+b2] -> tbl_mine
            if "A" in phases or "B" in phases:
             with tc.tile_pool(name="projA", bufs=3) as apool, \
                  tc.tile_pool(name="psumA", bufs=3, space="PSUM") as apsum:
                for t in range(NTILE_Y0):
                    r0 = t * P
                    r1 = min(NS, r0 + P)
                    w_ = r1 - r0
                    if w_ <= 0:
                        break
                    xt = apool.tile([F, P], f32, tag="xt")
                    nc.sync.dma_start(xt[:, :w_], xsT[:, r0:r1])
                    ps0 = apsum.tile([P, F], f32, space="PSUM")
                    nc.tensor.matmul(ps0[:w_, :], lhsT=xt[:, :w_],
                                     rhs=wt_t[0][:], start=True, stop=True)
                    st0 = apool.tile([P, F], f32, tag="st0")
                    nc.vector.tensor_tensor(out=st0[:w_, :], in0=ps0[:w_, :],
                                            in1=bb_t[0][:w_, :],
                                            op=mybir.AluOpType.add)
                    nc.sync.dma_start(y0_buf[r0:r1, :], st0[:w_, :])
                    st = apool.tile([P, 2 * F], f32, tag="st")
                    for j in (1, 2):
                        ps = apsum.tile([P, F], f32, space="PSUM")
                        nc.tensor.matmul(ps[:w_, :], lhsT=xt[:, :w_],
                                         rhs=wt_t[j][:], start=True, stop=True)
                        nc.vector.tensor_tensor(
                            out=st[:w_, (j - 1) * F:j * F], in0=ps[:w_, :],
                            in1=bb_t[j][:w_, :], op=mybir.AluOpType.add)
                    nc.sync.dma_start(tbl_mine[r0:r1, :], st[:w_, :])
             if "D" in phases:
                nc.gpsimd.collective_compute(
                    "AllGather", mybir.AluOpType.bypass,
                    replica_groups=[list(range(NC))],
                    ins=[tbl_mine[:]], outs=[table[:]],
                )

            # ---- Phase C: SpMM1 over table -> out1, z2s ------------------
            def spmm(src_w0, src_w1, fdim, dst_bufs):
                """One SpMM pass over the blocks.  dst_bufs: list of
                (dram_ap, col_offset) to scatter [P, F] slices of the psum."""
                qn = [0]
                with tc.tile_pool(name="ga", bufs=3) as gapool, \
                     tc.tile_pool(name="ix", bufs=3) as ixpool, \
                     tc.tile_pool(name="sS", bufs=4) as spool, \
                     tc.tile_pool(name="res", bufs=3) as rpool, \
                     tc.tile_pool(name="psC", bufs=4, space="PSUM") as cpsum:
                    for b in range(NBLK):
                        gts = []
                        for w, (Kw, Gw, src_w, idx_in) in enumerate(
                                [(K0, G0, src_w0, idx0_in),
                                 (K1, G1, src_w1, idx1_in)]):
                            for g in range(Gw):
                                cg = min(GM, Kw - GM * g)
                                r0 = (b * Gw + g) * P
                                ix = ixpool.tile([P, GM * 8], mybir.dt.int16,
                                                 tag=f"ix{w}_{g}")
                                nc.sync.dma_start(ix[:], idx_in[r0:r0 + P, :])
                                ga = gapool.tile([P, GM, fdim], f32,
                                                 tag=f"ga{w}_{g}")
                                nc.gpsimd.dma_gather(
                                    ga[:, :cg, :], src_w, ix[:, :cg * 8],
                                    num_idxs=cg * P, num_idxs_reg=cg * P,
                                    elem_size=fdim, queue_num=qn[0] % 4)
                                qn[0] += 1
                                gts.append(ga)
                        ps = cpsum.tile([P, fdim], f32, space="PSUM")
                        for k in range(K):
                            S = spool.tile([P, P], f32, tag="S")
                            mo = (b * K + k) * 2
                            nc.vector.tensor_scalar(
                                out=S[:], in0=iota_t[:],
                                scalar1=meta_t[:, mo:mo + 1],
                                scalar2=meta_t[:, mo + 1:mo + 2],
                                op0=mybir.AluOpType.is_equal,
                                op1=mybir.AluOpType.mult)
                            if k < K0:
                                rhs = gts[k // GM][:, k % GM, :]
                            else:
                                k1 = k - K0
                                rhs = gts[G0 + k1 // GM][:, k1 % GM, :]
                            nc.tensor.matmul(ps[:], lhsT=S[:], rhs=rhs,
                                             start=(k == 0),
                                             stop=(k == K - 1))
                        res = rpool.tile([P, fdim], f32, tag="res")
                        nc.vector.tensor_copy(res[:], ps[:])
                        if os.environ.get("KNOSCATTER"):
                            continue
                        for dst, coff in dst_bufs:
                            nc.gpsimd.indirect_dma_start(
                                out=dst,
                                out_offset=bass.IndirectOffsetOnAxis(
                                    ap=ids_t[:, b:b + 1], axis=0),
                                in_=res[:, coff:coff + F],
                                in_offset=None)

            if "C" in phases:
                spmm(table[:NW0, :], table[cfg.WIN:N, :], 2 * F,
                     [(out1_buf[:], 0), (z2s[:], F)])

            # ---- AllGather z2 shards ------------------------------------
            if "D" in phases:
                nc.gpsimd.collective_compute(
                    "AllGather", mybir.AluOpType.bypass,
                    replica_groups=[list(range(NC))],
                    ins=[z2s[0:NS, :]], outs=[z2t[:]],
                )

            # ---- Phase E: SpMM2 over z2 table -> out2 -------------------
            if "E" in phases:
                spmm(z2t[:NW0, :], z2t[cfg.WIN:N, :], F, [(out2_buf[:], 0)])

    nc.compile()
    return nc


_CACHE = {}


def _get_program(cfg, phases="ABCDE"):
    key = (cfg.N, cfg.F, cfg.NC, cfg.K0, cfg.K1, phases)
    if key not in _CACHE:
        _CACHE[key] = _build_program(cfg, phases)
    return _CACHE[key]


def _prepare(x, edge_weight, W, b, row, col, n_cores=8):
    N, F = x.shape
    row = np.asarray(row).astype(np.int64)
    col = np.asarray(col).astype(np.int64)
    w = np.asarray(edge_weight).astype(np.float32)
    x = np.asarray(x).astype(np.float32)
    W = np.asarray(W).astype(np.float32)
    b = np.asarray(b).astype(np.float32)

    ns = N // n_cores
    core_of = row // ns
    pcs = []
    for m in range(n_cores):
        sel = np.where(core_of == m)[0]
        cfg0 = Cfg(N, F, n_cores, 1, 1)
        pcs.append(_precompute_core(row[sel] - m * ns, col[sel], w[sel], cfg0))
    k0 = max(pc["k0"] for pc in pcs)
    k1 = max(pc["k1"] for pc in pcs)
    cfg = Cfg(N, F, n_cores, k0, k1)

    xT = np.ascontiguousarray(x.T)                       # [F, N]
    WT = np.ascontiguousarray(np.transpose(W, (0, 2, 1))).reshape(3 * F, F)
    BB = np.ascontiguousarray(np.broadcast_to(b[:, None, :], (3, P, F))).reshape(3 * P, F)
    iota = np.tile(np.arange(P, dtype=np.float32), (P, 1))

    in_maps = []
    for m in range(n_cores):
        enc = _encode_core(pcs[m], cfg)
        xs = np.zeros((F, cfg.NBLK * P), np.float32)
        xs[:, :ns] = xT[:, m * ns:(m + 1) * ns]
        in_maps.append(dict(
            xsT=xs, WT=WT, BB=BB, iota=iota,
            idx0=enc["idx0"], idx1=enc["idx1"], meta=enc["meta"],
            ids=enc["ids"],
        ))
    return cfg, in_maps


def kernel(x, edge_weight, W, b, row, col):
    n_cores = 8
    N, F = x.shape
    ns = N // n_cores
    cfg, in_maps = _prepare(x, edge_weight, W, b, row, col, n_cores)
    nc = _get_program(cfg)
    res = bass_utils.run_bass_kernel_spmd(nc, in_maps,
                                          core_ids=list(range(n_cores)))
    outs = []
    for m in range(n_cores):
        r = res.results[m]
        outs.append(np.concatenate(
            [r["y0"][:ns], r["out1"][:ns], r["out2"][:ns]], axis=1))
    return np.concatenate(outs, axis=0).astype(np.float32)


# revision 13
# speedup vs baseline: 1.4398x; 1.4398x over previous
"""MixHop layer (3 hops) on 8 Trainium2 NeuronCores.

out = concat_j [ adj_t^j @ (x @ W_j.T + b_j) ]   for j = 0,1,2

Strategy (destination sharding, one SPMD program on 8 cores):
  - Each core owns 6250 destination rows and the edges pointing into them.
  - Phase B: every core (redundantly) projects x -> [y1 | y2] table [N,256]
    with PE matmuls (x fed pre-transposed).  Phase A: y0 for own shard.
  - Phase C (SpMM1): per destination-block of 128 nodes (degree-balanced on
    host), dma_gather the 1024B table rows of each in-edge source, build the
    one-hot*weight segment matrix S on device (tensor_scalar is_equal+mult
    against an iota tile), segment-sum via PE matmuls accumulated in PSUM.
    Cols 0:128 -> out1 (scattered to output rows), cols 128:256 -> z2 shard.
  - AllGather z2 shards -> full z2 table [N,128].
  - Phase E (SpMM2): same edge structure gathers z2 -> out2.
All per-core variation (indices, segment matrices, scatter rows) is carried
as input data so a single program serves all cores.
"""

import sys

sys.path.insert(0, "/opt/trn_rl_repo")

import heapq
import os

import numpy as np

import concourse.bass as bass
import concourse.tile as tile
from concourse import bacc, mybir
from concourse import bass_utils

P = 128


class Cfg:
    def __init__(self, n_nodes, n_feat, n_cores, k0max, k1max, dt=mybir.dt.float32):
        assert n_nodes % n_cores == 0
        self.N = n_nodes
        self.F = n_feat          # 128
        self.NC = n_cores        # 8
        self.NS = n_nodes // n_cores          # dests per core
        self.NBLK = -(-self.NS // P)          # blocks per core
        if self.NBLK % 2:
            self.NBLK += 1                    # groups of 2 blocks
        self.NG = self.NBLK // 2
        self.GMAX = 8                         # chunks per dma_gather (<=1024 idxs)
        self.SGRP = 8                         # blocks per dma_scatter_add
        self.NSG = -(-self.NBLK // self.SGRP)
        self.G0 = -(-k0max // self.GMAX)
        self.G1 = -(-k1max // self.GMAX)
        self.K0 = k0max                       # window-0 chunks per block
        self.K1 = k1max                       # window-1 chunks per block
        self.K = k0max + k1max
        self.WIN = 32768 if n_nodes > 32768 else max(P, n_nodes // 2)
        self.NPAD = self.NBLK * P             # padded shard rows (incl trash)
        self.dt = dt


def _balanced_blocks(local_dest, ns, nblk):
    """Assign dests 0..ns-1 to nblk blocks of <=P slots, balancing edge counts.
    Returns (block_of[ns], pos_of[ns], ids[P, nblk] local out rows)."""
    deg = np.bincount(local_dest, minlength=ns)
    order = np.argsort(-deg, kind="stable")
    heap = [(0, 0, b) for b in range(nblk)]
    heapq.heapify(heap)
    block_of = np.empty(ns, np.int32)
    pos_of = np.empty(ns, np.int32)
    stash = []
    for d in order:
        while True:
            load, cnt, b = heapq.heappop(heap)
            if cnt < P:
                break
            stash.append((load, cnt, b))
        block_of[d] = b
        pos_of[d] = cnt
        heapq.heappush(heap, (load + int(deg[d]), cnt + 1, b))
    # ids: slot p of block b -> local output row (trash rows = ns + p for pads)
    ids = np.empty((P, nblk), np.int32)
    for p in range(P):
        ids[p, :] = ns + p  # trash default (distinct per slot within a block)
    ids[pos_of, block_of] = np.arange(ns, dtype=np.int32)
    return block_of, pos_of, ids


def _precompute_core(r_loc, c_glob, w, cfg):
    """Per-core edge streams.  Returns dict plus per-core chunk maxes."""
    ns, nblk = cfg.NS, cfg.NBLK
    block_of, pos_of, ids = _balanced_blocks(r_loc, ns, nblk)
    b_e = block_of[r_loc]
    dl_e = pos_of[r_loc]
    win_e = (c_glob >= cfg.WIN).astype(np.int64)
    # group edges by (block, window); stable order within group
    order = np.lexsort((np.arange(len(r_loc)), win_e, b_e))
    b_s, win_s, dl_s, c_s, w_s = (
        b_e[order], win_e[order], dl_e[order], c_glob[order], w[order])
    # counts per (block, window)
    key = b_s * 2 + win_s
    cnt = np.bincount(key, minlength=nblk * 2).reshape(nblk, 2)
    k0need = max(1, int(np.ceil(cnt[:, 0].max() / P))) if len(r_loc) else 1
    k1need = max(1, int(np.ceil(cnt[:, 1].max() / P))) if len(r_loc) else 1
    return dict(b=b_s, win=win_s, dl=dl_s, c=c_s, w=w_s, cnt=cnt, ids=ids,
                k0=k0need, k1=k1need)


def _encode_core(pc, cfg):
    """Build device input arrays for one core, given global K0/K1."""
    nblk, K0, K1, K = cfg.NBLK, cfg.K0, cfg.K1, cfg.K
    ng = cfg.NG
    cnt = pc["cnt"]
    # flat idx arrays, one per window, padded to static sizes (pad idx = 0)
    idx0 = np.zeros((nblk, K0 * P), np.int16)
    idx1 = np.zeros((nblk, K1 * P), np.int16)
    meta = np.zeros((P, nblk, K, 2), np.float32)
    # within-group position for every edge
    starts = np.zeros(nblk * 2, np.int64)
    starts[1:] = np.cumsum(cnt.reshape(-1))[:-1]
    key = pc["b"] * 2 + pc["win"]
    iw = np.arange(len(key)) - starts[key]          # index within (b, win)
    b, win, dl, c, w = pc["b"], pc["win"], pc["dl"], pc["c"], pc["w"]
    m0 = win == 0
    idx0[b[m0], iw[m0]] = c[m0].astype(np.int16)
    m1 = ~m0
    idx1[b[m1], iw[m1]] = (c[m1] - cfg.WIN).astype(np.int16)
    kk = np.where(m0, iw // P, K0 + iw // P)        # chunk within block
    meta[iw % P, b, kk, 0] = dl
    meta[iw % P, b, kk, 1] = w
    # per-gather idx encodings [nblk*Gw, 128, GMAX*8] (<=1024 idxs each)
    GM = cfg.GMAX
    def enc(idx, Kw, Gw):
        out = np.zeros((nblk, Gw, P, GM * 8), np.int16)
        for b in range(nblk):
            for g in range(Gw):
                cg = min(GM, Kw - GM * g)
                flat = idx[b, g * GM * P: g * GM * P + cg * P]
                e = flat.reshape(-1, 16).T                    # [16, cg*8]
                out[b, g, :, :cg * 8] = np.tile(e, (8, 1))
        return out.reshape(nblk * Gw * P, GM * 8)
    # batched scatter ids: group g covers blocks g*SGRP..; logical i -> row
    ids = pc["ids"]                                  # [P, nblk]
    sid = np.zeros((cfg.NSG, P, cfg.SGRP * 8), np.int16)
    for g in range(cfg.NSG):
        nb = min(cfg.SGRP, nblk - g * cfg.SGRP)
        flat = ids[:, g * cfg.SGRP: g * cfg.SGRP + nb].T.reshape(-1)  # i=c*128+p
        e = flat.reshape(-1, 16).T.astype(np.int16)
        sid[g, :, :nb * 8] = np.tile(e, (8, 1))
    return dict(
        idx0=enc(idx0, K0, cfg.G0), idx1=enc(idx1, K1, cfg.G1),
        meta=np.ascontiguousarray(meta.reshape(P, nblk * K * 2)),
        sid=sid.reshape(cfg.NSG * P, cfg.SGRP * 8),
    )


def _build_program(cfg, phases="ABCDE"):
    N, F, NC = cfg.N, cfg.F, cfg.NC
    NS, NBLK, NG, K0, K1, K = cfg.NS, cfg.NBLK, cfg.NG, cfg.K0, cfg.K1, cfg.K
    W0, W1 = 0, cfg.WIN                      # window bases
    NW0 = min(N, cfg.WIN)
    NW1 = max(0, N - cfg.WIN)
    NSP = NS + P                             # out buf rows incl trash
    NTILE_Y0 = NBLK                          # 128-row tiles for own shard
    NTILE_TAB = -(-N // P)                   # 128-row tiles for full table
    f32 = mybir.dt.float32
    f16 = mybir.dt.float16

    nc = bacc.Bacc("TRN2", target_bir_lowering=False, debug=False,
                   enable_asserts=False, num_devices=NC, num_swdge_queues=4)

    # ---- inputs -----------------------------------------------------------
    xsT = nc.dram_tensor("xsT", [F, NBLK * P], f32, kind="ExternalInput").ap()
    WT = nc.dram_tensor("WT", [3 * F, F], f32, kind="ExternalInput").ap()
    BB = nc.dram_tensor("BB", [3 * P, F], f32, kind="ExternalInput").ap()
    iota_in = nc.dram_tensor("iota", [P, P], f32, kind="ExternalInput").ap()
    GM, G0, G1 = cfg.GMAX, cfg.G0, cfg.G1
    idx0_in = nc.dram_tensor("idx0", [NBLK * G0 * P, GM * 8], mybir.dt.int16,
                             kind="ExternalInput").ap()
    idx1_in = nc.dram_tensor("idx1", [NBLK * G1 * P, GM * 8], mybir.dt.int16,
                             kind="ExternalInput").ap()
    meta_in = nc.dram_tensor("meta", [P, NBLK * K * 2], f32,
                             kind="ExternalInput").ap()
    SG, NSG = cfg.SGRP, cfg.NSG
    sid_in = nc.dram_tensor("sid", [NSG * P, SG * 8], mybir.dt.int16,
                            kind="ExternalInput").ap()

    # ---- outputs / scratch ------------------------------------------------
    y0_buf = nc.dram_tensor("y0", [NBLK * P, F], f32, kind="ExternalOutput").ap()
    out1_buf = nc.dram_tensor("out1", [NSP, F], f32, kind="ExternalOutput").ap()
    out2_buf = nc.dram_tensor("out2", [NSP, F], f32, kind="ExternalOutput").ap()
    tbl_mine = nc.dram_tensor("tbl_mine", [NS, 2 * F], f16, kind="Internal").ap()
    table = nc.dram_tensor("table", [N, 2 * F], f16, kind="Internal",
                           addr_space="Shared").ap()
    z2s = nc.dram_tensor("z2s", [NSP, F], f16, kind="Internal").ap()
    z2t = nc.dram_tensor("z2t", [N, F], f16, kind="Internal",
                         addr_space="Shared").ap()

    with tile.TileContext(nc) as tc:
        with tc.tile_pool(name="const", bufs=1) as cpool:
            iota_t = cpool.tile([P, P], f32)
            nc.sync.dma_start(iota_t[:], iota_in[:])

            meta_t = cpool.tile([P, NBLK * K * 2], f32)
            nc.sync.dma_start(meta_t[:], meta_in[:])
            wt_t = []
            bb_t = []
            for j in range(3):
                wtj = cpool.tile([F, F], f32, tag=f"wt{j}", name=f"wt{j}")
                bbj = cpool.tile([P, F], f32, tag=f"bb{j}", name=f"bb{j}")
                wt_t.append(wtj)
                bb_t.append(bbj)
            for j in range(3):
                nc.sync.dma_start(wt_t[j][:], WT[j * F:(j + 1) * F, :])
                nc.sync.dma_start(bb_t[j][:], BB[j * P:(j + 1) * P, :])

            # ---- zero z2s (scatter-add base) -----------------------------
            if "C" in phases:
                with tc.tile_pool(name="zz", bufs=1) as zpool:
                    zt = zpool.tile([P, 2048], f16)
                    nc.vector.memset(zt[:], 0.0)
                    nrow = 0
                    while nrow + 2048 <= NSP:
                        nc.sync.dma_start(
                            z2s[nrow:nrow + 2048, :].rearrange(
                                "(a b) f -> a (b f)", a=P), zt[:])
                        nrow += 2048
                    while nrow + P <= NSP:
                        nc.sync.dma_start(
                            z2s[nrow:nrow + P, :].rearrange(
                                "(a b) f -> a (b f)", a=P), zt[:, :F])
                        nrow += P
                    assert nrow >= NS, (nrow, NS)

            # ---- Phase A/B: own-shard projections ------------------------
            # y0 = xs@W0.T+b0 -> y0_buf; [xs@W1.T+b1 | xs@W2.T+b2] -> tbl_mine
            if "A" in phases or "B" in phases:
             with tc.tile_pool(name="projA", bufs=3) as apool, \
                  tc.tile_pool(name="psumA", bufs=3, space="PSUM") as apsum:
                for t in range(NTILE_Y0):
                    r0 = t * P
                    r1 = min(NS, r0 + P)
                    w_ = r1 - r0
                    if w_ <= 0:
                        break
                    xt = apool.tile([F, P], f32, tag="xt")
                    nc.sync.dma_start(xt[:, :w_], xsT[:, r0:r1])
                    ps0 = apsum.tile([P, F], f32, space="PSUM")
                    nc.tensor.matmul(ps0[:w_, :], lhsT=xt[:, :w_],
                                     rhs=wt_t[0][:], start=True, stop=True)
                    st0 = apool.tile([P, F], f32, tag="st0")
                    nc.vector.tensor_tensor(out=st0[:w_, :], in0=ps0[:w_, :],
                                            in1=bb_t[0][:w_, :],
                                            op=mybir.AluOpType.add)
                    nc.sync.dma_start(y0_buf[r0:r1, :], st0[:w_, :])
                    st = apool.tile([P, 2 * F], f16, tag="st")
                    for j in (1, 2):
                        ps = apsum.tile([P, F], f32, space="PSUM")
                        nc.tensor.matmul(ps[:w_, :], lhsT=xt[:, :w_],
                                         rhs=wt_t[j][:], start=True, stop=True)
                        nc.vector.tensor_tensor(
                            out=st[:w_, (j - 1) * F:j * F], in0=ps[:w_, :],
                            in1=bb_t[j][:w_, :], op=mybir.AluOpType.add)
                    nc.sync.dma_start(tbl_mine[r0:r1, :], st[:w_, :])
             if "D" in phases:
                nc.gpsimd.collective_compute(
                    "AllGather", mybir.AluOpType.bypass,
                    replica_groups=[list(range(NC))],
                    ins=[tbl_mine[:]], outs=[table[:]],
                )

            # ---- Phase C: SpMM1 over table -> out1, z2s ------------------
            def spmm(src_w0, src_w1, fdim, dst_bufs, gdt, stg_dts):
                """One SpMM pass over the blocks.  dst_bufs: list of
                (dram_ap, col_offset) receiving [P, F] slices of the psum
                via batched dma_scatter_add into pre-zeroed buffers."""
                qn = [0]
                with tc.tile_pool(name="ga", bufs=3) as gapool, \
                     tc.tile_pool(name="ix", bufs=3) as ixpool, \
                     tc.tile_pool(name="sS", bufs=4) as spool, \
                     tc.tile_pool(name="stg", bufs=2) as stgpool, \
                     tc.tile_pool(name="psC", bufs=4, space="PSUM") as cpsum:
                    stgs = None
                    for b in range(NBLK):
                        g_s, c_s = b // SG, b % SG
                        nb = min(SG, NBLK - g_s * SG)
                        if c_s == 0:
                            stgs = [stgpool.tile([P, SG, F], stg_dts[i],
                                                 tag=f"stg{i}", name=f"stg{i}_{g_s}")
                                    for i in range(len(dst_bufs))]
                            sid_t = stgpool.tile([P, SG * 8], mybir.dt.int16,
                                                 tag="sid", name=f"sid_{g_s}")
                            nc.sync.dma_start(sid_t[:],
                                              sid_in[g_s * P:(g_s + 1) * P, :])
                        gts = []
                        for w, (Kw, Gw, src_w, idx_in) in enumerate(
                                [(K0, G0, src_w0, idx0_in),
                                 (K1, G1, src_w1, idx1_in)]):
                            for g in range(Gw):
                                cg = min(GM, Kw - GM * g)
                                r0 = (b * Gw + g) * P
                                ix = ixpool.tile([P, GM * 8], mybir.dt.int16,
                                                 tag=f"ix{w}_{g}")
                                nc.sync.dma_start(ix[:], idx_in[r0:r0 + P, :])
                                ga = gapool.tile([P, GM, fdim], gdt,
                                                 tag=f"ga{w}_{g}")
                                nc.gpsimd.dma_gather(
                                    ga[:, :cg, :], src_w, ix[:, :cg * 8],
                                    num_idxs=cg * P, num_idxs_reg=cg * P,
                                    elem_size=fdim, queue_num=0)
                                qn[0] += 1
                                gts.append(ga)
                        ps = cpsum.tile([P, fdim], f32, space="PSUM")
                        for k in range(K):
                            S = spool.tile([P, P], gdt, tag="S")
                            mo = (b * K + k) * 2
                            nc.vector.tensor_scalar(
                                out=S[:], in0=iota_t[:],
                                scalar1=meta_t[:, mo:mo + 1],
                                scalar2=meta_t[:, mo + 1:mo + 2],
                                op0=mybir.AluOpType.is_equal,
                                op1=mybir.AluOpType.mult)
                            if k < K0:
                                rhs = gts[k // GM][:, k % GM, :]
                            else:
                                k1 = k - K0
                                rhs = gts[G0 + k1 // GM][:, k1 % GM, :]
                            nc.tensor.matmul(ps[:], lhsT=S[:], rhs=rhs,
                                             start=(k == 0),
                                             stop=(k == K - 1))
                        for i, (dst, coff) in enumerate(dst_bufs):
                            nc.vector.tensor_copy(stgs[i][:, c_s, :],
                                                  ps[:, coff:coff + F])
                        if c_s == nb - 1:
                            for i, (dst, coff) in enumerate(dst_bufs):
                                nc.gpsimd.dma_scatter_add(
                                    dst, stgs[i][:, :nb, :],
                                    sid_t[:, :nb * 8],
                                    num_idxs=nb * P, num_idxs_reg=nb * P,
                                    elem_size=F, queue_num=0)
                                qn[0] += 1

            if "C" in phases:
                spmm(table[:NW0, :], table[cfg.WIN:N, :], 2 * F,
                     [(out1_buf[:], 0), (z2s[:], F)], f16, [f32, f16])

            # ---- AllGather z2 shards ------------------------------------
            if "D" in phases:
                nc.gpsimd.collective_compute(
                    "AllGather", mybir.AluOpType.bypass,
                    replica_groups=[list(range(NC))],
                    ins=[z2s[0:NS, :]], outs=[z2t[:]],
                )

            # ---- Phase E: SpMM2 over z2 table -> out2 -------------------
            if "E" in phases:
                spmm(z2t[:NW0, :], z2t[cfg.WIN:N, :], F, [(out2_buf[:], 0)],
                     f16, [f32])

    nc.compile()
    return nc


_CACHE = {}


def _get_program(cfg, phases="ABCDE"):
    key = (cfg.N, cfg.F, cfg.NC, cfg.K0, cfg.K1, phases)
    if key not in _CACHE:
        _CACHE[key] = _build_program(cfg, phases)
    return _CACHE[key]


def _prepare(x, edge_weight, W, b, row, col, n_cores=8):
    N, F = x.shape
    row = np.asarray(row).astype(np.int64)
    col = np.asarray(col).astype(np.int64)
    w = np.asarray(edge_weight).astype(np.float32)
    x = np.asarray(x).astype(np.float32)
    W = np.asarray(W).astype(np.float32)
    b = np.asarray(b).astype(np.float32)

    ns = N // n_cores
    core_of = row // ns
    pcs = []
    for m in range(n_cores):
        sel = np.where(core_of == m)[0]
        cfg0 = Cfg(N, F, n_cores, 1, 1)
        pcs.append(_precompute_core(row[sel] - m * ns, col[sel], w[sel], cfg0))
    k0 = max(pc["k0"] for pc in pcs)
    k1 = max(pc["k1"] for pc in pcs)
    cfg = Cfg(N, F, n_cores, k0, k1)

    xT = np.ascontiguousarray(x.T)                       # [F, N]
    WT = np.ascontiguousarray(np.transpose(W, (0, 2, 1))).reshape(3 * F, F)
    BB = np.ascontiguousarray(np.broadcast_to(b[:, None, :], (3, P, F))).reshape(3 * P, F)
    iota = np.tile(np.arange(P, dtype=np.float32), (P, 1))

    in_maps = []
    for m in range(n_cores):
        enc = _encode_core(pcs[m], cfg)
        xs = np.zeros((F, cfg.NBLK * P), np.float32)
        xs[:, :ns] = xT[:, m * ns:(m + 1) * ns]
        in_maps.append(dict(
            xsT=xs, WT=WT, BB=BB, iota=iota,
            idx0=enc["idx0"], idx1=enc["idx1"], meta=enc["meta"],
            sid=enc["sid"],
        ))
    return cfg, in_maps


def kernel(x, edge_weight, W, b, row, col):
    n_cores = 8
    N, F = x.shape
    ns = N // n_cores
    cfg, in_maps = _prepare(x, edge_weight, W, b, row, col, n_cores)
    nc = _get_program(cfg)
    res = bass_utils.run_bass_kernel_spmd(nc, in_maps,
                                          core_ids=list(range(n_cores)))
    outs = []
    for m in range(n_cores):
        r = res.results[m]
        outs.append(np.concatenate(
            [r["y0"][:ns], r["out1"][:ns], r["out2"][:ns]], axis=1))
    return np.concatenate(outs, axis=0).astype(np.float32)
